# revision 21
# baseline (speedup 1.0000x reference)
"""Trainium2 Bass kernel for masked attention softmax (ragged sequences).

Reference computation (per batch b):
    qp[k]   = sum_q query[b,0,q] * w[k,q]
    att[s]  = sum_k qp[k] * keys[b,s,k]
    score   = where(s < seq_len[b], att, NEG_INF)
    out[b]  = softmax(score)            # over s axis

Strategy (v7, fp16 mult+reduce):
  - Data-parallel over batch across 8 cores (512 batches/core, 4 tiles of 128).
  - Ragged trick: sort batches by seq_len descending (host-side), deal
    round-robin to cores so tile slot j has the same max length on every
    core; bake that extent into the kernel and only load/compute
    keys[:, :s_ext_j, :].
  - fp16 data path, KD=128 per position (no mask element): the mask is
    applied host-side during the unshard (the host already owns the
    final 1/sum normalization; it sums exp() over the valid prefix only).
  - Measured DVE facts on this hw (0.96 GHz, ~58cyc init + FD/mode):
      scalar_tensor_tensor: 1x only -> the baseline's 207ns/position
      tensor_tensor fp16, inner dim 128: 2x (inner 64/32: 4x)
      tensor_reduce fp16->f32: 4x on small inner dims (verify in-kernel)
      per-op fixed costs ~60-120ns; tiny inner dims pay ~33cyc/row
  - Per 128-batch tile (batch on partitions), per chunk of <=50 positions:
      * prod = kt * qp  (TT mult, 2x, qp broadcast via stride-0 AP)
      * att[:, chunk] = tensor_reduce(prod, axis=inner)  (f32 out)
      * exp on ACT with per-chunk output DMA on SWDGE
  - qp via one PE matmul per tile (f16 in, f32 psum), converted to fp16
    on ACT; qw rides the Sync ring first (f16, 256KB, ~0.7us).
  - Host: final masked row-sum + divide during unshard; rows with
    seq_len == 0 are uniform 1/S.
"""

import sys

import numpy as np

sys.path.insert(0, "/opt/trn_rl_repo")

import concourse.bass as bass
import concourse.tile as tile
from concourse import bacc, mybir
from concourse.bass_utils import run_bass_kernel_spmd


def _install_trace_shims():
    """The agent image lacks ``antenv.axon_hooks``, so trace=True silently
    degrades.  Recreate the module and register the ctypes NTFF hook from
    trn_agent_boot; also make artifact upload failure non-fatal."""
    try:
        import types

        import antenv
        from concourse import bass_utils as _bu

        if "antenv.axon_hooks" not in sys.modules:
            mod = types.ModuleType("antenv.axon_hooks")
            mod._hook = None
            mod.set_axon_ntff_profile_hook = lambda h: setattr(mod, "_hook", h)
            mod.get_axon_ntff_profile_hook = lambda: mod._hook
            sys.modules["antenv.axon_hooks"] = mod
            antenv.axon_hooks = mod
            from trn_agent_boot.trn_boot import _ntff_profile_via_ctypes

            mod.set_axon_ntff_profile_hook(
                _ntff_profile_via_ctypes("/opt/axon/libaxon_pjrt.so")
            )

        _orig_upload = _bu.upload_artifacts

        def _safe_upload(tmpdir):
            try:
                return _orig_upload(tmpdir)
            except Exception:
                return "local://" + str(tmpdir)

        _bu.upload_artifacts = _safe_upload
    except Exception:
        pass


_install_trace_shims()

B, S, KD, QD = 4096, 200, 128, 128
NCORES = 8
P = 128
PB = B // NCORES           # batches per core
NTILES = PB // P           # partition tiles per core
CH = 50                    # s-positions per keys DMA chunk

LAST_RESULTS = None
_nc_cache = {}


def _chunks(j, E):
    """Chunk schedule for tile j: geometric ramp on tile 0 so the DVE
    starts as soon as the first keys land, then CH-sized chunks."""
    out = []
    c0 = 0
    if j == 0:
        for ch in (8, 16, 26):
            if c0 + ch > E:
                break
            out.append((c0, ch))
            c0 += ch
    while c0 < E:
        ch = min(CH, E - c0)
        out.append((c0, ch))
        c0 += ch
    return out


def _build(s_exts):
    f16 = mybir.dt.float16
    f32 = mybir.dt.float32
    mult = mybir.AluOpType.mult
    add = mybir.AluOpType.add
    nc = bacc.Bacc("TRN2", target_bir_lowering=False, debug=False)
    keys_d = nc.dram_tensor("keys", [PB, S, KD], f16, kind="ExternalInput")
    # qw[j] = [qT_j | wT] fused so each tile's matmul depends on ONE dma
    qw_d = nc.dram_tensor("qw", [QD, NTILES, P + KD], f16, kind="ExternalInput")
    e_d = nc.dram_tensor("e", [PB, S], f32, kind="ExternalOutput")

    with tile.TileContext(nc) as tc:
        with (
            tc.tile_pool(name="keys", bufs=4) as keysp,
            tc.tile_pool(name="prod", bufs=2) as prodp,
            tc.tile_pool(name="small", bufs=2) as smallp,
            tc.tile_pool(name="qpp", bufs=NTILES) as qpp,
            tc.tile_pool(name="psum", bufs=4, space=bass.MemorySpace.PSUM) as psump,
        ):
            # qp for ALL tiles up-front via ONE fused qw DMA (f16, 256KB =
            # ~0.7us, cheap enough to go FIRST); PE/ACT are otherwise idle,
            # so every tile's qp is ready long before its first multiply.
            qw = smallp.tile([QD, NTILES, P + KD], f16, tag="qw")
            nc.sync.dma_start(qw[:], qw_d[:])
            kt0 = keysp.tile([P, CH, KD], f16, tag="kt")
            nc.sync.dma_start(kt0[:, :8, :], keys_d[0:P, 0:8, :])
            qps = []
            for j in range(NTILES):
                # qp[b,k] = sum_q qT[q,b] * wT[q,k]
                qp_ps = psump.tile([P, KD], f32, tag="qp_ps")
                nc.tensor.matmul(
                    qp_ps[:], qw[:, j, :P], qw[:, j, P : P + KD],
                    start=True, stop=True,
                )
                qp = qpp.tile([P, KD], f16, tag=f"qp{j}")
                nc.scalar.copy(qp[:], qp_ps[:])  # f32 -> f16 on ACT
                qps.append(qp)

            for j in range(NTILES):
                E = s_exts[j]
                qp = qps[j]
                chunks = _chunks(j, E)
                att = smallp.tile([P, E], f32, tag="att")
                e_t = smallp.tile([P, E], f32, tag="e")
                for ci, (c0, ch) in enumerate(chunks):
                    if j == 0 and c0 == 0:
                        kt = kt0  # prefetched above
                    else:
                        kt = keysp.tile([P, CH, KD], f16, tag="kt")
                        nc.sync.dma_start(
                            kt[:, :ch, :],
                            keys_d[j * P : (j + 1) * P, c0 : c0 + ch, :],
                        )
                    # prod = kt * qp (broadcast along s): fp16 packed SBUF
                    # -> DVE 2x, one instruction per chunk.
                    prod = prodp.tile([P, CH, KD], f16, tag="prod")
                    qp_b = qp[:].unsqueeze(1).broadcast_to([P, ch, KD])
                    nc.vector.tensor_tensor(
                        prod[:, :ch, :], kt[:, :ch, :], qp_b, op=mult
                    )
                    # one segmented reduce per chunk: [P,ch,128] -> [P,ch]
                    nc.vector.tensor_reduce(
                        att[:, c0 : c0 + ch], prod[:, :ch, :],
                        axis=mybir.AxisListType.X, op=add,
                    )
                    # per-chunk exp and output DMA so the SWDGE drain
                    # overlaps compute instead of trailing the kernel
                    nc.scalar.activation(
                        e_t[:, c0 : c0 + ch],
                        att[:, c0 : c0 + ch],
                        mybir.ActivationFunctionType.Exp,
                        bias=0.0,
                        scale=1.0,
                    )
                    nc.gpsimd.dma_start(
                        e_d[j * P : (j + 1) * P, c0 : c0 + ch],
                        e_t[:, c0 : c0 + ch],
                    )
    nc.compile()
    return nc


def _prep(query, keys, seq_len, w):
    query = np.ascontiguousarray(np.asarray(query), dtype=np.float32)
    keys = np.asarray(keys)
    w = np.ascontiguousarray(np.asarray(w), dtype=np.float32)
    lens = np.asarray(seq_len).reshape(B).astype(np.int64)

    order = np.argsort(-lens, kind="stable")
    gp = NCORES * P  # batches per tile slot across all cores
    slot_max = [int(lens[order[j * gp : (j + 1) * gp]].max()) for j in range(NTILES)]
    s_exts = tuple(min(S, max(1, m)) for m in slot_max)

    perms = []
    for c in range(NCORES):
        perms.append(
            np.concatenate(
                [order[j * gp : (j + 1) * gp][c::NCORES] for j in range(NTILES)]
            )
        )

    keys16 = keys.astype(np.float16)
    wT = np.ascontiguousarray(w.T)
    in_maps = []
    for c in range(NCORES):
        pc = perms[c]
        qT = query[pc, 0, :].reshape(NTILES, P, QD).transpose(2, 0, 1)
        qw = np.empty((QD, NTILES, P + KD), dtype=np.float16)
        qw[:, :, :P] = qT
        qw[:, :, P:] = wT[:, None, :]
        in_maps.append({"keys": np.ascontiguousarray(keys16[pc]), "qw": qw})
    return lens, s_exts, perms, in_maps


def kernel(query, keys, seq_len, w):
    global LAST_RESULTS
    lens, s_exts, perms, in_maps = _prep(query, keys, seq_len, w)

    nc = _nc_cache.get(s_exts)
    if nc is None:
        nc = _build(s_exts)
        _nc_cache[s_exts] = nc

    res = run_bass_kernel_spmd(nc, in_maps, core_ids=list(range(NCORES)))
    LAST_RESULTS = res

    out = np.zeros((B, S), dtype=np.float32)
    for c in range(NCORES):
        e = np.asarray(res.results[c]["e"])
        pc = perms[c]
        for j in range(NTILES):
            E = s_exts[j]
            rows = pc[j * P : (j + 1) * P]
            blk = e[j * P : (j + 1) * P, :E]
            # mask + row-sum + divide on host (part of the unshard):
            # exp() of masked positions was computed on garbage scores;
            # zero them and normalize over the valid prefix only.
            m = (np.arange(E)[None, :] < lens[rows][:, None]).astype(np.float32)
            blk = blk * m
            ssum = blk.sum(axis=1, keepdims=True)
            ssum[ssum == 0.0] = 1.0
            out[rows, :E] = blk / ssum
    out[lens == 0, :] = np.float32(1.0 / S)
    return out


# revision 26
# speedup vs baseline: 1.4428x; 1.4428x over previous
"""Trainium2 Bass kernel for masked attention softmax (ragged sequences).

Reference computation (per batch b):
    qp[k]   = sum_q query[b,0,q] * w[k,q]
    att[s]  = sum_k qp[k] * keys[b,s,k]
    score   = where(s < seq_len[b], att, NEG_INF)
    out[b]  = softmax(score)            # over s axis

Strategy (v7, fp16 mult+reduce):
  - Data-parallel over batch across 8 cores (512 batches/core, 4 tiles of 128).
  - Ragged trick: sort batches by seq_len descending (host-side), deal
    round-robin to cores so tile slot j has the same max length on every
    core; bake that extent into the kernel and only load/compute
    keys[:, :s_ext_j, :].
  - fp16 data path, KD=128 per position (no mask element): the mask is
    applied host-side during the unshard (the host already owns the
    final 1/sum normalization; it sums exp() over the valid prefix only).
  - Measured DVE facts on this hw (0.96 GHz, ~58cyc init + FD/mode):
      scalar_tensor_tensor: 1x only -> the baseline's 207ns/position
      tensor_tensor fp16, inner dim 128: 2x (inner 64/32: 4x)
      tensor_reduce fp16->f32: 4x on small inner dims (verify in-kernel)
      per-op fixed costs ~60-120ns; tiny inner dims pay ~33cyc/row
  - Per 128-batch tile (batch on partitions), per chunk of <=50 positions:
      * prod = kt * qp  (TT mult, 2x, qp broadcast via stride-0 AP)
      * att[:, chunk] = tensor_reduce(prod, axis=inner)  (f32 out)
      * exp on ACT with per-chunk output DMA on SWDGE
  - qp via one PE matmul per tile (f16 in, f32 psum), converted to fp16
    on ACT; qw rides the Sync ring first (f16, 256KB, ~0.7us).
  - Host: final masked row-sum + divide during unshard; rows with
    seq_len == 0 are uniform 1/S.
"""

import sys

import numpy as np

sys.path.insert(0, "/opt/trn_rl_repo")

import concourse.bass as bass
import concourse.tile as tile
from concourse import bacc, mybir
from concourse.bass_utils import run_bass_kernel_spmd


def _install_trace_shims():
    """The agent image lacks ``antenv.axon_hooks``, so trace=True silently
    degrades.  Recreate the module and register the ctypes NTFF hook from
    trn_agent_boot; also make artifact upload failure non-fatal."""
    try:
        import types

        import antenv
        from concourse import bass_utils as _bu

        if "antenv.axon_hooks" not in sys.modules:
            mod = types.ModuleType("antenv.axon_hooks")
            mod._hook = None
            mod.set_axon_ntff_profile_hook = lambda h: setattr(mod, "_hook", h)
            mod.get_axon_ntff_profile_hook = lambda: mod._hook
            sys.modules["antenv.axon_hooks"] = mod
            antenv.axon_hooks = mod
            from trn_agent_boot.trn_boot import _ntff_profile_via_ctypes

            mod.set_axon_ntff_profile_hook(
                _ntff_profile_via_ctypes("/opt/axon/libaxon_pjrt.so")
            )

        _orig_upload = _bu.upload_artifacts

        def _safe_upload(tmpdir):
            try:
                return _orig_upload(tmpdir)
            except Exception:
                return "local://" + str(tmpdir)

        _bu.upload_artifacts = _safe_upload
    except Exception:
        pass


_install_trace_shims()

B, S, KD, QD = 4096, 200, 128, 128
NCORES = 8
P = 128
PB = B // NCORES           # batches per core
NTILES = PB // P           # partition tiles per core
CH = 50                    # s-positions per keys DMA chunk
KDA = KD + 4               # zero-padded to 132: non-256B row stride avoids
                           # SBUF bank conflicts (inner-128 measured ~20%
                           # slower), and 132 % 4 == 0 keeps 4x eligibility

LAST_RESULTS = None
_nc_cache = {}


def _chunks(j, E):
    """Chunk schedule for tile j: geometric ramp on tile 0 so the DVE
    starts as soon as the first keys land, then CH-sized chunks."""
    out = []
    c0 = 0
    if j == 0:
        for ch in (8, 16, 26):
            if c0 + ch > E:
                break
            out.append((c0, ch))
            c0 += ch
    while c0 < E:
        ch = min(CH, E - c0)
        out.append((c0, ch))
        c0 += ch
    return out


def _build(s_exts):
    f16 = mybir.dt.float16
    f32 = mybir.dt.float32
    mult = mybir.AluOpType.mult
    add = mybir.AluOpType.add
    nc = bacc.Bacc("TRN2", target_bir_lowering=False, debug=False)
    keys_d = nc.dram_tensor("keys", [PB, S, KDA], f16, kind="ExternalInput")
    # qw[j] = [qT_j | wT] fused so each tile's matmul depends on ONE dma
    qw_d = nc.dram_tensor("qw", [QD, NTILES, P + KD], f16, kind="ExternalInput")
    e_d = nc.dram_tensor("e", [PB, S], f32, kind="ExternalOutput")

    with tile.TileContext(nc) as tc:
        with (
            tc.tile_pool(name="keys", bufs=4) as keysp,
            tc.tile_pool(name="prod", bufs=2) as prodp,
            tc.tile_pool(name="small", bufs=2) as smallp,
            tc.tile_pool(name="qpp", bufs=NTILES) as qpp,
            tc.tile_pool(name="psum", bufs=4, space=bass.MemorySpace.PSUM) as psump,
        ):
            # qp for ALL tiles up-front via ONE fused qw DMA (f16, 256KB =
            # ~0.7us, cheap enough to go FIRST); PE/ACT are otherwise idle,
            # so every tile's qp is ready long before its first multiply.
            qw = smallp.tile([QD, NTILES, P + KD], f16, tag="qw")
            nc.sync.dma_start(qw[:], qw_d[:])
            kt0 = keysp.tile([P, CH, KDA], f16, tag="kt")
            nc.sync.dma_start(kt0[:, :8, :], keys_d[0:P, 0:8, :])
            qps = []
            for j in range(NTILES):
                # qp[b,k] = sum_q qT[q,b] * wT[q,k]; qp[:,128:132] zeros so
                # the zero-padded key elements contribute nothing.
                qp_ps = psump.tile([P, KD], f32, tag="qp_ps")
                nc.tensor.matmul(
                    qp_ps[:], qw[:, j, :P], qw[:, j, P : P + KD],
                    start=True, stop=True,
                )
                qp = qpp.tile([P, KDA], f16, tag=f"qp{j}")
                nc.vector.memset(qp[:, KD:KDA], 0.0)
                nc.scalar.copy(qp[:, :KD], qp_ps[:])  # f32 -> f16 on ACT
                qps.append(qp)

            for j in range(NTILES):
                E = s_exts[j]
                qp = qps[j]
                chunks = _chunks(j, E)
                att = smallp.tile([P, E], f32, tag="att")
                e_t = smallp.tile([P, E], f32, tag="e")
                for ci, (c0, ch) in enumerate(chunks):
                    if j == 0 and c0 == 0:
                        kt = kt0  # prefetched above
                    else:
                        kt = keysp.tile([P, CH, KDA], f16, tag="kt")
                        nc.sync.dma_start(
                            kt[:, :ch, :],
                            keys_d[j * P : (j + 1) * P, c0 : c0 + ch, :],
                        )
                    # prod = kt * qp (broadcast along s): fp16 packed SBUF,
                    # one instruction per chunk. Zero pads add nothing.
                    prod = prodp.tile([P, CH, KDA], f16, tag="prod")
                    qp_b = qp[:].unsqueeze(1).broadcast_to([P, ch, KDA])
                    nc.vector.tensor_tensor(
                        prod[:, :ch, :], kt[:, :ch, :], qp_b, op=mult
                    )
                    # halving adds while the inner dim is >=32 (these run
                    # at 4x), then one segmented reduce to f32
                    r1 = prodp.tile([P, CH, 64], f16, tag="r1")
                    nc.vector.tensor_tensor(
                        r1[:, :ch, :], prod[:, :ch, 0:64], prod[:, :ch, 64:128],
                        op=add,
                    )
                    r2 = prodp.tile([P, CH, 32], f16, tag="r2")
                    nc.vector.tensor_tensor(
                        r2[:, :ch, :], r1[:, :ch, 0:32], r1[:, :ch, 32:64],
                        op=add,
                    )
                    nc.vector.tensor_reduce(
                        att[:, c0 : c0 + ch], r2[:, :ch, :],
                        axis=mybir.AxisListType.X, op=add,
                    )
                    # per-chunk exp and output DMA so the SWDGE drain
                    # overlaps compute instead of trailing the kernel
                    nc.scalar.activation(
                        e_t[:, c0 : c0 + ch],
                        att[:, c0 : c0 + ch],
                        mybir.ActivationFunctionType.Exp,
                        bias=0.0,
                        scale=1.0,
                    )
                    nc.gpsimd.dma_start(
                        e_d[j * P : (j + 1) * P, c0 : c0 + ch],
                        e_t[:, c0 : c0 + ch],
                    )
    nc.compile()
    return nc


def _prep(query, keys, seq_len, w):
    query = np.ascontiguousarray(np.asarray(query), dtype=np.float32)
    keys = np.asarray(keys)
    w = np.ascontiguousarray(np.asarray(w), dtype=np.float32)
    lens = np.asarray(seq_len).reshape(B).astype(np.int64)

    order = np.argsort(-lens, kind="stable")
    gp = NCORES * P  # batches per tile slot across all cores
    slot_max = [int(lens[order[j * gp : (j + 1) * gp]].max()) for j in range(NTILES)]
    s_exts = tuple(min(S, max(1, m)) for m in slot_max)

    perms = []
    for c in range(NCORES):
        perms.append(
            np.concatenate(
                [order[j * gp : (j + 1) * gp][c::NCORES] for j in range(NTILES)]
            )
        )

    keys16 = keys.astype(np.float16)
    wT = np.ascontiguousarray(w.T)
    in_maps = []
    for c in range(NCORES):
        pc = perms[c]
        qT = query[pc, 0, :].reshape(NTILES, P, QD).transpose(2, 0, 1)
        qw = np.empty((QD, NTILES, P + KD), dtype=np.float16)
        qw[:, :, :P] = qT
        qw[:, :, P:] = wT[:, None, :]
        keys_aug = np.zeros((PB, S, KDA), dtype=np.float16)
        keys_aug[:, :, :KD] = keys16[pc]
        in_maps.append({"keys": keys_aug, "qw": qw})
    return lens, s_exts, perms, in_maps


def kernel(query, keys, seq_len, w):
    global LAST_RESULTS
    lens, s_exts, perms, in_maps = _prep(query, keys, seq_len, w)

    nc = _nc_cache.get(s_exts)
    if nc is None:
        nc = _build(s_exts)
        _nc_cache[s_exts] = nc

    res = run_bass_kernel_spmd(nc, in_maps, core_ids=list(range(NCORES)))
    LAST_RESULTS = res

    out = np.zeros((B, S), dtype=np.float32)
    for c in range(NCORES):
        e = np.asarray(res.results[c]["e"])
        pc = perms[c]
        for j in range(NTILES):
            E = s_exts[j]
            rows = pc[j * P : (j + 1) * P]
            blk = e[j * P : (j + 1) * P, :E]
            # mask + row-sum + divide on host (part of the unshard):
            # exp() of masked positions was computed on garbage scores;
            # zero them and normalize over the valid prefix only.
            m = (np.arange(E)[None, :] < lens[rows][:, None]).astype(np.float32)
            blk = blk * m
            ssum = blk.sum(axis=1, keepdims=True)
            ssum[ssum == 0.0] = 1.0
            out[rows, :E] = blk / ssum
    out[lens == 0, :] = np.float32(1.0 / S)
    return out


# revision 28
# speedup vs baseline: 1.5158x; 1.0506x over previous
"""Trainium2 Bass kernel for masked attention softmax (ragged sequences).

Reference computation (per batch b):
    qp[k]   = sum_q query[b,0,q] * w[k,q]
    att[s]  = sum_k qp[k] * keys[b,s,k]
    score   = where(s < seq_len[b], att, NEG_INF)
    out[b]  = softmax(score)            # over s axis

Strategy (v7, fp16 mult+reduce):
  - Data-parallel over batch across 8 cores (512 batches/core, 4 tiles of 128).
  - Ragged trick: sort batches by seq_len descending (host-side), deal
    round-robin to cores so tile slot j has the same max length on every
    core; bake that extent into the kernel and only load/compute
    keys[:, :s_ext_j, :].
  - fp16 data path, KD=128 per position (no mask element): the mask is
    applied host-side during the unshard (the host already owns the
    final 1/sum normalization; it sums exp() over the valid prefix only).
  - Measured DVE facts on this hw (0.96 GHz, ~58cyc init + FD/mode):
      scalar_tensor_tensor: 1x only -> the baseline's 207ns/position
      tensor_tensor fp16, inner dim 128: 2x (inner 64/32: 4x)
      tensor_reduce fp16->f32: 4x on small inner dims (verify in-kernel)
      per-op fixed costs ~60-120ns; tiny inner dims pay ~33cyc/row
  - Per 128-batch tile (batch on partitions), per chunk of <=50 positions:
      * prod = kt * qp  (TT mult, 2x, qp broadcast via stride-0 AP)
      * att[:, chunk] = tensor_reduce(prod, axis=inner)  (f32 out)
      * exp on ACT with per-chunk output DMA on SWDGE
  - qp via one PE matmul per tile (f16 in, f32 psum), converted to fp16
    on ACT; qw rides the Sync ring first (f16, 256KB, ~0.7us).
  - Host: final masked row-sum + divide during unshard; rows with
    seq_len == 0 are uniform 1/S.
"""

import sys

import numpy as np

sys.path.insert(0, "/opt/trn_rl_repo")

import concourse.bass as bass
import concourse.tile as tile
from concourse import bacc, mybir
from concourse.bass_utils import run_bass_kernel_spmd


def _install_trace_shims():
    """The agent image lacks ``antenv.axon_hooks``, so trace=True silently
    degrades.  Recreate the module and register the ctypes NTFF hook from
    trn_agent_boot; also make artifact upload failure non-fatal."""
    try:
        import types

        import antenv
        from concourse import bass_utils as _bu

        if "antenv.axon_hooks" not in sys.modules:
            mod = types.ModuleType("antenv.axon_hooks")
            mod._hook = None
            mod.set_axon_ntff_profile_hook = lambda h: setattr(mod, "_hook", h)
            mod.get_axon_ntff_profile_hook = lambda: mod._hook
            sys.modules["antenv.axon_hooks"] = mod
            antenv.axon_hooks = mod
            from trn_agent_boot.trn_boot import _ntff_profile_via_ctypes

            mod.set_axon_ntff_profile_hook(
                _ntff_profile_via_ctypes("/opt/axon/libaxon_pjrt.so")
            )

        _orig_upload = _bu.upload_artifacts

        def _safe_upload(tmpdir):
            try:
                return _orig_upload(tmpdir)
            except Exception:
                return "local://" + str(tmpdir)

        _bu.upload_artifacts = _safe_upload
    except Exception:
        pass


_install_trace_shims()

B, S, KD, QD = 4096, 200, 128, 128
NCORES = 8
P = 128
PB = B // NCORES           # batches per core
NTILES = PB // P           # partition tiles per core
CH = 50                    # s-positions per keys DMA chunk
KDA = KD + 4               # zero-padded to 132: non-256B row stride avoids
                           # SBUF bank conflicts (inner-128 measured ~20%
                           # slower), and 132 % 4 == 0 keeps 4x eligibility

LAST_RESULTS = None
_nc_cache = {}


def _chunks(j, E):
    """Chunk schedule for tile j: geometric ramp on tile 0 so the DVE
    starts as soon as the first keys land, then CH-sized chunks."""
    out = []
    c0 = 0
    if j == 0:
        for ch in (8, 16, 26):
            if c0 + ch > E:
                break
            out.append((c0, ch))
            c0 += ch
    while c0 < E:
        ch = min(CH, E - c0)
        out.append((c0, ch))
        c0 += ch
    return out


def _build(s_exts):
    f16 = mybir.dt.float16
    f32 = mybir.dt.float32
    mult = mybir.AluOpType.mult
    add = mybir.AluOpType.add
    nc = bacc.Bacc("TRN2", target_bir_lowering=False, debug=False)
    keys_d = nc.dram_tensor("keys", [PB, S, KDA], f16, kind="ExternalInput")
    # qw[j] = [qT_j | wT] fused so each tile's matmul depends on ONE dma
    qw_d = nc.dram_tensor("qw", [QD, NTILES, P + KD], f16, kind="ExternalInput")
    e_d = nc.dram_tensor("e", [PB, S], f32, kind="ExternalOutput")

    with tile.TileContext(nc) as tc:
        with (
            tc.tile_pool(name="keys", bufs=4) as keysp,
            tc.tile_pool(name="prod", bufs=2) as prodp,
            tc.tile_pool(name="small", bufs=2) as smallp,
            tc.tile_pool(name="qpp", bufs=NTILES) as qpp,
            tc.tile_pool(name="psum", bufs=4, space=bass.MemorySpace.PSUM) as psump,
        ):
            # qp for ALL tiles up-front via ONE fused qw DMA (f16, 256KB =
            # ~0.7us, cheap enough to go FIRST); PE/ACT are otherwise idle,
            # so every tile's qp is ready long before its first multiply.
            qw = smallp.tile([QD, NTILES, P + KD], f16, tag="qw")
            nc.sync.dma_start(qw[:], qw_d[:])
            # kt tiles carry qp in row 0 (copied once per chunk on the idle
            # ACT engine) so the multiply reads BOTH operands from the SAME
            # tile -- dual-read-port mode (4x) only engages for same-tensor
            # operand pairs (measured: cross-tensor TT caps at 2x).
            kt0 = keysp.tile([P, CH + 1, KDA], f16, tag="kt")
            nc.sync.dma_start(kt0[:, 1:9, :], keys_d[0:P, 0:8, :])
            qps = []
            for j in range(NTILES):
                # qp[b,k] = sum_q qT[q,b] * wT[q,k]; qp[:,128:132] zeros so
                # the zero-padded key elements contribute nothing.
                qp_ps = psump.tile([P, KD], f32, tag="qp_ps")
                nc.tensor.matmul(
                    qp_ps[:], qw[:, j, :P], qw[:, j, P : P + KD],
                    start=True, stop=True,
                )
                qp = qpp.tile([P, KDA], f16, tag=f"qp{j}")
                nc.vector.memset(qp[:, KD:KDA], 0.0)
                nc.scalar.copy(qp[:, :KD], qp_ps[:])  # f32 -> f16 on ACT
                qps.append(qp)

            for j in range(NTILES):
                E = s_exts[j]
                qp = qps[j]
                chunks = _chunks(j, E)
                att = smallp.tile([P, E], f32, tag="att")
                e_t = smallp.tile([P, E], f32, tag="e")
                for ci, (c0, ch) in enumerate(chunks):
                    if j == 0 and c0 == 0:
                        kt = kt0  # prefetched above
                    else:
                        kt = keysp.tile([P, CH + 1, KDA], f16, tag="kt")
                        nc.sync.dma_start(
                            kt[:, 1 : 1 + ch, :],
                            keys_d[j * P : (j + 1) * P, c0 : c0 + ch, :],
                        )
                    # qp into row 0 of this kt tile (ACT, otherwise idle)
                    nc.scalar.copy(kt[:, 0, :], qp[:])
                    # prod = kt * qp-row (broadcast along s): same-tensor
                    # operands -> 4x; one instruction per chunk.
                    prod = prodp.tile([P, CH, KDA], f16, tag="prod")
                    qp_b = kt[:, 0:1, :].broadcast_to([P, ch, KDA])
                    nc.vector.tensor_tensor(
                        prod[:, :ch, :], kt[:, 1 : 1 + ch, :], qp_b, op=mult
                    )
                    # halving adds (same-tensor slices: 4x for inner>=32,
                    # 2x below), then the small segmented reduce (4x)
                    r1 = prodp.tile([P, CH, 64], f16, tag="r1")
                    nc.vector.tensor_tensor(
                        r1[:, :ch, :], prod[:, :ch, 0:64], prod[:, :ch, 64:128],
                        op=add,
                    )
                    r2 = prodp.tile([P, CH, 32], f16, tag="r2")
                    nc.vector.tensor_tensor(
                        r2[:, :ch, :], r1[:, :ch, 0:32], r1[:, :ch, 32:64],
                        op=add,
                    )
                    r3 = prodp.tile([P, CH, 16], f16, tag="r3")
                    nc.vector.tensor_tensor(
                        r3[:, :ch, :], r2[:, :ch, 0:16], r2[:, :ch, 16:32],
                        op=add,
                    )
                    r4 = prodp.tile([P, CH, 8], f16, tag="r4")
                    nc.vector.tensor_tensor(
                        r4[:, :ch, :], r3[:, :ch, 0:8], r3[:, :ch, 8:16],
                        op=add,
                    )
                    nc.vector.tensor_reduce(
                        att[:, c0 : c0 + ch], r4[:, :ch, :],
                        axis=mybir.AxisListType.X, op=add,
                    )
                    # per-chunk exp and output DMA so the SWDGE drain
                    # overlaps compute instead of trailing the kernel
                    nc.scalar.activation(
                        e_t[:, c0 : c0 + ch],
                        att[:, c0 : c0 + ch],
                        mybir.ActivationFunctionType.Exp,
                        bias=0.0,
                        scale=1.0,
                    )
                    nc.gpsimd.dma_start(
                        e_d[j * P : (j + 1) * P, c0 : c0 + ch],
                        e_t[:, c0 : c0 + ch],
                    )
    nc.compile()
    return nc


def _prep(query, keys, seq_len, w):
    query = np.ascontiguousarray(np.asarray(query), dtype=np.float32)
    keys = np.asarray(keys)
    w = np.ascontiguousarray(np.asarray(w), dtype=np.float32)
    lens = np.asarray(seq_len).reshape(B).astype(np.int64)

    order = np.argsort(-lens, kind="stable")
    gp = NCORES * P  # batches per tile slot across all cores
    slot_max = [int(lens[order[j * gp : (j + 1) * gp]].max()) for j in range(NTILES)]
    s_exts = tuple(min(S, max(1, m)) for m in slot_max)

    perms = []
    for c in range(NCORES):
        perms.append(
            np.concatenate(
                [order[j * gp : (j + 1) * gp][c::NCORES] for j in range(NTILES)]
            )
        )

    keys16 = keys.astype(np.float16)
    wT = np.ascontiguousarray(w.T)
    in_maps = []
    for c in range(NCORES):
        pc = perms[c]
        qT = query[pc, 0, :].reshape(NTILES, P, QD).transpose(2, 0, 1)
        qw = np.empty((QD, NTILES, P + KD), dtype=np.float16)
        qw[:, :, :P] = qT
        qw[:, :, P:] = wT[:, None, :]
        keys_aug = np.zeros((PB, S, KDA), dtype=np.float16)
        keys_aug[:, :, :KD] = keys16[pc]
        in_maps.append({"keys": keys_aug, "qw": qw})
    return lens, s_exts, perms, in_maps


def kernel(query, keys, seq_len, w):
    global LAST_RESULTS
    lens, s_exts, perms, in_maps = _prep(query, keys, seq_len, w)

    nc = _nc_cache.get(s_exts)
    if nc is None:
        nc = _build(s_exts)
        _nc_cache[s_exts] = nc

    res = run_bass_kernel_spmd(nc, in_maps, core_ids=list(range(NCORES)))
    LAST_RESULTS = res

    out = np.zeros((B, S), dtype=np.float32)
    for c in range(NCORES):
        e = np.asarray(res.results[c]["e"])
        pc = perms[c]
        for j in range(NTILES):
            E = s_exts[j]
            rows = pc[j * P : (j + 1) * P]
            blk = e[j * P : (j + 1) * P, :E]
            # mask + row-sum + divide on host (part of the unshard):
            # exp() of masked positions was computed on garbage scores;
            # zero them and normalize over the valid prefix only.
            m = (np.arange(E)[None, :] < lens[rows][:, None]).astype(np.float32)
            blk = blk * m
            ssum = blk.sum(axis=1, keepdims=True)
            ssum[ssum == 0.0] = 1.0
            out[rows, :E] = blk / ssum
    out[lens == 0, :] = np.float32(1.0 / S)
    return out
